# revision 10
# baseline (speedup 1.0000x reference)
"""Trainium2 Bass kernel for nn_BackwardTransformLayer (inverse wavelet step).

Math (polyphase form of the reference):
    g = flip(scaling_rec); g[1::2] *= -1
    E[r, u] = sum_{j=0..3} g[2j]   * d[r, (u+j)   % M] + s[2j]   * a[r, (u+j)   % M]
    O[r, u] = sum_{j=0..3} g[2j+1] * d[r, (u+1+j) % M] + s[2j+1] * a[r, (u+1+j) % M]
    out[r, 2u] = E[r, u]; out[r, 2u+1] = O[r, u]

Harness tolerance is 2e-2, so everything runs in fp16 (rel err ~5e-4).

Layout trick: inputs are transposed HOST-side so the FIR axis u lies on SBUF
partitions.  A single matmul with a banded 128x128 weight matrix
W[k, p] = coef[k-p] computes a full 4-tap FIR for 124 output columns in one
pass over the moving tensor (rows in the free dim):

    psE[p, r] = sum_k Wd_e[k, p] * dT[k, r] + Wa_e[k, p] * aT[k, r]

Four matmuls per 124-column tile chunk (d/a x even/odd polyphase) do all 16
MACs/column, so PE covers the WHOLE problem (~62us/core).  ScalarE and
VectorE split the PSUM -> SBUF fp16 drains.  The kernel is DMA-bound.

DMA: d/a are interleaved host-side into xT[u, 2, r] and E/O outputs into
oT[u, 2, r], so each 128-column tile is ONE 2MB load and ONE 2MB store
(8KB contiguous lines).  Loads/stores alternate between the two HWDGE
rings (SP / ACT) per tile to split bytes evenly across both rings.

Sharding: embarrassingly parallel over columns u: 1024 columns per core
(+4 circular halo), all 4096 rows in the free dim.
"""

import numpy as np

P = 128
M = 8192                       # input columns (output cols = 2M interleaved)
N_ROWS = 4096
N_CORES = 8
COLS_PER_CORE = M // N_CORES   # 1024
HALO = 4                       # odd polyphase reaches k = p+4
STRIDE = P - HALO              # 124 valid output columns per 128-partition tile
RCHUNK = 512                   # PSUM bank capacity in f32
NRCH = N_ROWS // RCHUNK        # 8 row chunks
_CACHE = {}


def _tiles():
    """(col_offset, k_width, p_width) per tile covering [0, COLS_PER_CORE)."""
    out = []
    p0 = 0
    while p0 < COLS_PER_CORE:
        pw = min(STRIDE, COLS_PER_CORE - p0)
        kw = min(pw + HALO, P)
        out.append((p0, kw, pw))
        p0 += pw
    return out


def _build(reps=1):
    import contextlib

    import concourse.bacc as bacc
    import concourse.mybir as mybir
    from concourse.tile import TileContext

    f32 = mybir.dt.float32
    f16 = mybir.dt.float16

    nc = bacc.Bacc("TRN2", target_bir_lowering=False, debug=False)
    xT = nc.dram_tensor(
        "xT", [COLS_PER_CORE + HALO, 2, N_ROWS], f16, kind="ExternalInput"
    )
    wb = nc.dram_tensor("wb", [P, 4 * P], f16, kind="ExternalInput")
    oT = nc.dram_tensor("oT", [COLS_PER_CORE, 2, N_ROWS], f16, kind="ExternalOutput")

    with TileContext(nc) as tc:
        with (
            tc.tile_pool(name="const", bufs=1) as const_pool,
            tc.tile_pool(name="tin", bufs=4) as tin_pool,
            tc.tile_pool(name="tout", bufs=3) as tout_pool,
            tc.tile_pool(name="psum", bufs=4, space="PSUM") as psum_pool,
        ):
            wb_sb = const_pool.tile([P, 4 * P], f16)
            nc.sync.dma_start(out=wb_sb[:], in_=wb[:])
            # weight blocks: 0=Wd_even 1=Wa_even 2=Wd_odd 3=Wa_odd
            W = [wb_sb[:, b * P:(b + 1) * P] for b in range(4)]

            rep_ctx = tc.For_i(0, reps, 1) if reps > 1 else contextlib.nullcontext()
            with rep_ctx:
                for ti, (p0, kw, pw) in enumerate(_tiles()):
                    ld_eng, st_eng = (
                        (nc.sync, nc.scalar) if ti % 2 == 0 else (nc.scalar, nc.sync)
                    )
                    # x_t holds d in cols [0, N_ROWS), a in [N_ROWS, 2*N_ROWS)
                    x_t = tin_pool.tile([P, 2 * N_ROWS], f16, tag="x")
                    ld_eng.dma_start(
                        out=x_t[:kw], in_=xT[p0:p0 + kw].rearrange("k i r -> k (i r)")
                    )
                    d_t = x_t[:, 0:N_ROWS]
                    a_t = x_t[:, N_ROWS:2 * N_ROWS]

                    # o_t holds E in cols [0, N_ROWS), O in [N_ROWS, 2*N_ROWS)
                    o_t = tout_pool.tile([P, 2 * N_ROWS], f16, tag="o")
                    for c in range(NRCH):
                        r0 = c * RCHUNK
                        rs = slice(r0, r0 + RCHUNK)
                        psE = psum_pool.tile([P, RCHUNK], f32, tag="psE")
                        psO = psum_pool.tile([P, RCHUNK], f32, tag="psO")
                        nc.tensor.matmul(
                            psE[:pw], W[0][:kw, :pw], d_t[:kw, rs],
                            start=True, stop=False,
                        )
                        nc.tensor.matmul(
                            psE[:pw], W[1][:kw, :pw], a_t[:kw, rs],
                            start=False, stop=True,
                        )
                        nc.tensor.matmul(
                            psO[:pw], W[2][:kw, :pw], d_t[:kw, rs],
                            start=True, stop=False,
                        )
                        nc.tensor.matmul(
                            psO[:pw], W[3][:kw, :pw], a_t[:kw, rs],
                            start=False, stop=True,
                        )
                        # split the PSUM drains across ScalarE and VectorE
                        nc.scalar.copy(o_t[:pw, r0:r0 + RCHUNK], psE[:pw])
                        nc.vector.tensor_copy(
                            o_t[:pw, N_ROWS + r0:N_ROWS + r0 + RCHUNK], psO[:pw]
                        )
                    st_eng.dma_start(
                        out=oT[p0:p0 + pw].rearrange("k i r -> k (i r)"), in_=o_t[:pw]
                    )
    nc.compile()
    return nc


def _prep_inputs(details, approximation, scaling, scaling_rec):
    d = np.asarray(details, dtype=np.float32)
    a = np.asarray(approximation, dtype=np.float32)
    s = np.asarray(scaling, dtype=np.float64)
    sr = np.asarray(scaling_rec, dtype=np.float64)

    g = sr[::-1].copy()
    g[1::2] *= -1.0

    # banded weights W[k, p] = coef[k - p]
    wb_np = np.zeros((P, 4 * P), np.float16)
    kk = np.arange(P)[:, None]
    pp = np.arange(P)[None, :]
    diff = kk - pp
    for b, (filt, lo) in enumerate(((g, 0), (s, 0), (g, 1), (s, 1))):
        # even blocks (lo=0): coef[j] = filt[2j], j = k-p in [0, 3]
        # odd blocks (lo=1):  coef[j] = filt[2j+1], j = k-p-1 in [0, 3]
        j = diff - lo
        mask = (j >= 0) & (j < 4)
        vals = np.zeros((P, P), np.float32)
        vals[mask] = np.asarray(filt, np.float32)[2 * j[mask] + lo]
        wb_np[:, b * P:(b + 1) * P] = vals.astype(np.float16)

    # xT[u, 0, r] = d[r, u]; xT[u, 1, r] = a[r, u]  (u has 4 wrap columns)
    dTf = np.concatenate([d, d[:, :HALO]], axis=1).astype(np.float16).T
    aTf = np.concatenate([a, a[:, :HALO]], axis=1).astype(np.float16).T
    xT_np = np.ascontiguousarray(np.stack([dTf, aTf], axis=1))
    return xT_np, wb_np


def make_in_maps(details, approximation, scaling, scaling_rec):
    xT_np, wb_np = _prep_inputs(details, approximation, scaling, scaling_rec)
    in_maps = []
    for core in range(N_CORES):
        u0 = core * COLS_PER_CORE
        u1 = u0 + COLS_PER_CORE + HALO
        in_maps.append({"xT": xT_np[u0:u1], "wb": wb_np})
    return in_maps


def kernel(details, approximation, scaling, scaling_rec):
    if "nc" not in _CACHE:
        _CACHE["nc"] = _build()
    nc = _CACHE["nc"]

    from concourse.bass_utils import run_bass_kernel_spmd

    in_maps = make_in_maps(details, approximation, scaling, scaling_rec)
    res = run_bass_kernel_spmd(nc, in_maps, core_ids=list(range(N_CORES)))
    oT = np.concatenate([r["oT"] for r in res.results], axis=0)  # [M, 2, N_ROWS]
    out = np.empty((N_ROWS, 2 * M), np.float32)
    out[:, 0::2] = oT[:, 0].T
    out[:, 1::2] = oT[:, 1].T
    return out


# revision 11
# speedup vs baseline: 1.4790x; 1.4790x over previous
"""Trainium2 Bass kernel for nn_BackwardTransformLayer (inverse wavelet step).

Math (polyphase form of the reference):
    g = flip(scaling_rec); g[1::2] *= -1
    E[r, u] = sum_{j=0..3} g[2j]   * d[r, (u+j)   % M] + s[2j]   * a[r, (u+j)   % M]
    O[r, u] = sum_{j=0..3} g[2j+1] * d[r, (u+1+j) % M] + s[2j+1] * a[r, (u+1+j) % M]
    out[r, 2u] = E[r, u]; out[r, 2u+1] = O[r, u]

Harness tolerance is 2e-2, so everything runs in fp16 (rel err ~5e-4).

Layout trick: inputs are transposed HOST-side so the FIR axis u lies on SBUF
partitions.  A single matmul with a banded 128x128 weight matrix
W[k, p] = coef[k-p] computes a full 4-tap FIR for 124 output columns in one
pass over the moving tensor (rows in the free dim):

    psE[p, r] = sum_k Wd_e[k, p] * dT[k, r] + Wa_e[k, p] * aT[k, r]

Four matmuls per 124-column tile chunk (d/a x even/odd polyphase) do all 16
MACs/column, so PE covers the WHOLE problem (~62us/core).  ScalarE and
VectorE split the PSUM -> SBUF fp16 drains.  The kernel is DMA-bound.

DMA: d/a are interleaved host-side into xT[u, 2, r] and E/O outputs into
oT[u, 2, r], so each 128-column tile is ONE 2MB load and ONE 2MB store
(8KB contiguous lines).  Loads/stores alternate between the two HWDGE
rings (SP / ACT) per tile to split bytes evenly across both rings.

Sharding: embarrassingly parallel over columns u: 1024 columns per core
(+4 circular halo), all 4096 rows in the free dim.
"""

import numpy as np

P = 128
M = 8192                       # input columns (output cols = 2M interleaved)
N_ROWS = 4096
N_CORES = 8
COLS_PER_CORE = M // N_CORES   # 1024
HALO = 4                       # odd polyphase reaches k = p+4
STRIDE = P - HALO              # 124 valid output columns per 128-partition tile
RCHUNK = 512                   # PSUM bank capacity in f32
NRCH = N_ROWS // RCHUNK        # 8 row chunks
_CACHE = {}


def _tiles():
    """(col_offset, k_width, p_width) per tile covering [0, COLS_PER_CORE)."""
    out = []
    p0 = 0
    while p0 < COLS_PER_CORE:
        pw = min(STRIDE, COLS_PER_CORE - p0)
        kw = min(pw + HALO, P)
        out.append((p0, kw, pw))
        p0 += pw
    return out


def _build(reps=1):
    import contextlib

    import concourse.bacc as bacc
    import concourse.mybir as mybir
    from concourse.tile import TileContext

    f32 = mybir.dt.float32
    f16 = mybir.dt.float16

    nc = bacc.Bacc("TRN2", target_bir_lowering=False, debug=False)
    xT = nc.dram_tensor(
        "xT", [COLS_PER_CORE + HALO, 2, N_ROWS], f16, kind="ExternalInput"
    )
    wb = nc.dram_tensor("wb", [P, 4 * P], f16, kind="ExternalInput")
    oT = nc.dram_tensor("oT", [COLS_PER_CORE, 2, N_ROWS], f16, kind="ExternalOutput")

    with TileContext(nc) as tc:
        with (
            tc.tile_pool(name="const", bufs=1) as const_pool,
            tc.tile_pool(name="tin", bufs=4) as tin_pool,
            tc.tile_pool(name="tout", bufs=3) as tout_pool,
            tc.tile_pool(name="psum", bufs=4, space="PSUM") as psum_pool,
        ):
            wb_sb = const_pool.tile([P, 4 * P], f16)
            nc.sync.dma_start(out=wb_sb[:], in_=wb[:])
            # weight blocks: 0=Wd_even 1=Wa_even 2=Wd_odd 3=Wa_odd
            W = [wb_sb[:, b * P:(b + 1) * P] for b in range(4)]

            rep_ctx = tc.For_i(0, reps, 1) if reps > 1 else contextlib.nullcontext()
            with rep_ctx:
                for ti, (p0, kw, pw) in enumerate(_tiles()):
                    # Loads on the SP HWDGE ring (SP runs nothing else);
                    # stores on the GPSIMD SWDGE ring (also otherwise idle).
                    # ACT/DVE only drain PSUM, so no DMA trigger ever queues
                    # behind a compute wait.
                    ld_eng, st_eng = nc.sync, nc.gpsimd
                    # x_t holds d in cols [0, N_ROWS), a in [N_ROWS, 2*N_ROWS)
                    x_t = tin_pool.tile([P, 2 * N_ROWS], f16, tag="x")
                    ld_eng.dma_start(
                        out=x_t[:kw], in_=xT[p0:p0 + kw].rearrange("k i r -> k (i r)")
                    )
                    d_t = x_t[:, 0:N_ROWS]
                    a_t = x_t[:, N_ROWS:2 * N_ROWS]

                    # o_t holds E in cols [0, N_ROWS), O in [N_ROWS, 2*N_ROWS)
                    o_t = tout_pool.tile([P, 2 * N_ROWS], f16, tag="o")
                    for c in range(NRCH):
                        r0 = c * RCHUNK
                        rs = slice(r0, r0 + RCHUNK)
                        psE = psum_pool.tile([P, RCHUNK], f32, tag="psE")
                        psO = psum_pool.tile([P, RCHUNK], f32, tag="psO")
                        nc.tensor.matmul(
                            psE[:pw], W[0][:kw, :pw], d_t[:kw, rs],
                            start=True, stop=False,
                        )
                        nc.tensor.matmul(
                            psE[:pw], W[1][:kw, :pw], a_t[:kw, rs],
                            start=False, stop=True,
                        )
                        nc.tensor.matmul(
                            psO[:pw], W[2][:kw, :pw], d_t[:kw, rs],
                            start=True, stop=False,
                        )
                        nc.tensor.matmul(
                            psO[:pw], W[3][:kw, :pw], a_t[:kw, rs],
                            start=False, stop=True,
                        )
                        # split the PSUM drains across ScalarE and VectorE
                        nc.scalar.copy(o_t[:pw, r0:r0 + RCHUNK], psE[:pw])
                        nc.vector.tensor_copy(
                            o_t[:pw, N_ROWS + r0:N_ROWS + r0 + RCHUNK], psO[:pw]
                        )
                    st_eng.dma_start(
                        out=oT[p0:p0 + pw].rearrange("k i r -> k (i r)"), in_=o_t[:pw]
                    )
    nc.compile()
    return nc


def _prep_inputs(details, approximation, scaling, scaling_rec):
    d = np.asarray(details, dtype=np.float32)
    a = np.asarray(approximation, dtype=np.float32)
    s = np.asarray(scaling, dtype=np.float64)
    sr = np.asarray(scaling_rec, dtype=np.float64)

    g = sr[::-1].copy()
    g[1::2] *= -1.0

    # banded weights W[k, p] = coef[k - p]
    wb_np = np.zeros((P, 4 * P), np.float16)
    kk = np.arange(P)[:, None]
    pp = np.arange(P)[None, :]
    diff = kk - pp
    for b, (filt, lo) in enumerate(((g, 0), (s, 0), (g, 1), (s, 1))):
        # even blocks (lo=0): coef[j] = filt[2j], j = k-p in [0, 3]
        # odd blocks (lo=1):  coef[j] = filt[2j+1], j = k-p-1 in [0, 3]
        j = diff - lo
        mask = (j >= 0) & (j < 4)
        vals = np.zeros((P, P), np.float32)
        vals[mask] = np.asarray(filt, np.float32)[2 * j[mask] + lo]
        wb_np[:, b * P:(b + 1) * P] = vals.astype(np.float16)

    # xT[u, 0, r] = d[r, u]; xT[u, 1, r] = a[r, u]  (u has 4 wrap columns)
    dTf = np.concatenate([d, d[:, :HALO]], axis=1).astype(np.float16).T
    aTf = np.concatenate([a, a[:, :HALO]], axis=1).astype(np.float16).T
    xT_np = np.ascontiguousarray(np.stack([dTf, aTf], axis=1))
    return xT_np, wb_np


def make_in_maps(details, approximation, scaling, scaling_rec):
    xT_np, wb_np = _prep_inputs(details, approximation, scaling, scaling_rec)
    in_maps = []
    for core in range(N_CORES):
        u0 = core * COLS_PER_CORE
        u1 = u0 + COLS_PER_CORE + HALO
        in_maps.append({"xT": xT_np[u0:u1], "wb": wb_np})
    return in_maps


def kernel(details, approximation, scaling, scaling_rec):
    if "nc" not in _CACHE:
        _CACHE["nc"] = _build()
    nc = _CACHE["nc"]

    from concourse.bass_utils import run_bass_kernel_spmd

    in_maps = make_in_maps(details, approximation, scaling, scaling_rec)
    res = run_bass_kernel_spmd(nc, in_maps, core_ids=list(range(N_CORES)))
    oT = np.concatenate([r["oT"] for r in res.results], axis=0)  # [M, 2, N_ROWS]
    out = np.empty((N_ROWS, 2 * M), np.float32)
    out[:, 0::2] = oT[:, 0].T
    out[:, 1::2] = oT[:, 1].T
    return out


# revision 12
# speedup vs baseline: 1.6357x; 1.1060x over previous
"""Trainium2 Bass kernel for nn_BackwardTransformLayer (inverse wavelet step).

Math (polyphase form of the reference):
    g = flip(scaling_rec); g[1::2] *= -1
    E[r, u] = sum_{j=0..3} g[2j]   * d[r, (u+j)   % M] + s[2j]   * a[r, (u+j)   % M]
    O[r, u] = sum_{j=0..3} g[2j+1] * d[r, (u+1+j) % M] + s[2j+1] * a[r, (u+1+j) % M]
    out[r, 2u] = E[r, u]; out[r, 2u+1] = O[r, u]

Harness tolerance is 2e-2, so everything runs in fp16 (rel err ~5e-4).

Layout trick: inputs are transposed HOST-side so the FIR axis u lies on SBUF
partitions.  A single matmul with a banded 128x128 weight matrix
W[k, p] = coef[k-p] computes a full 4-tap FIR for 124 output columns in one
pass over the moving tensor (rows in the free dim):

    psE[p, r] = sum_k Wd_e[k, p] * dT[k, r] + Wa_e[k, p] * aT[k, r]

Four matmuls per 124-column tile chunk (d/a x even/odd polyphase) do all 16
MACs/column, so PE covers the WHOLE problem (~62us/core).  ScalarE and
VectorE split the PSUM -> SBUF fp16 drains.  The kernel is DMA-bound.

DMA: d/a are interleaved host-side into xT[u, 2, r] and E/O outputs into
oT[u, 2, r], so each 128-column tile is ONE 2MB load and ONE 2MB store
(8KB contiguous lines).  Loads/stores alternate between the two HWDGE
rings (SP / ACT) per tile to split bytes evenly across both rings.

Sharding: embarrassingly parallel over columns u: 1024 columns per core
(+4 circular halo), all 4096 rows in the free dim.
"""

import numpy as np

P = 128
M = 8192                       # input columns (output cols = 2M interleaved)
N_ROWS = 4096
N_CORES = 8
COLS_PER_CORE = M // N_CORES   # 1024
HALO = 4                       # odd polyphase reaches k = p+4
STRIDE = P - HALO              # 124 valid output columns per 128-partition tile
RCHUNK = 512                   # PSUM bank capacity in f32
NRCH = N_ROWS // RCHUNK        # 8 row chunks
_CACHE = {}


def _tiles():
    """(col_offset, k_width, p_width) per tile covering [0, COLS_PER_CORE)."""
    out = []
    p0 = 0
    while p0 < COLS_PER_CORE:
        pw = min(STRIDE, COLS_PER_CORE - p0)
        kw = min(pw + HALO, P)
        out.append((p0, kw, pw))
        p0 += pw
    return out


def _build(reps=1):
    import contextlib

    import concourse.bacc as bacc
    import concourse.mybir as mybir
    from concourse.tile import TileContext

    f32 = mybir.dt.float32
    f16 = mybir.dt.float16

    nc = bacc.Bacc("TRN2", target_bir_lowering=False, debug=False)
    xT = nc.dram_tensor(
        "xT", [COLS_PER_CORE + HALO, 2, N_ROWS], f16, kind="ExternalInput"
    )
    wb = nc.dram_tensor("wb", [P, 4 * P], f16, kind="ExternalInput")
    oT = nc.dram_tensor("oT", [COLS_PER_CORE, 2, N_ROWS], f16, kind="ExternalOutput")

    with TileContext(nc) as tc:
        with (
            tc.tile_pool(name="const", bufs=1) as const_pool,
            tc.tile_pool(name="tin", bufs=6) as tin_pool,
            tc.tile_pool(name="tout", bufs=4) as tout_pool,
            tc.tile_pool(name="psum", bufs=4, space="PSUM") as psum_pool,
        ):
            wb_sb = const_pool.tile([P, 4 * P], f16)
            nc.sync.dma_start(out=wb_sb[:], in_=wb[:])
            # weight blocks: 0=Wd_even 1=Wa_even 2=Wd_odd 3=Wa_odd
            W = [wb_sb[:, b * P:(b + 1) * P] for b in range(4)]

            rep_ctx = tc.For_i(0, reps, 1) if reps > 1 else contextlib.nullcontext()
            with rep_ctx:
                for ti, (p0, kw, pw) in enumerate(_tiles()):
                    # Loads on the SP HWDGE ring (SP runs nothing else);
                    # stores on the GPSIMD SWDGE ring (also otherwise idle).
                    # ACT/DVE only drain PSUM, so no DMA trigger ever queues
                    # behind a compute wait.
                    ld_eng, st_eng = nc.sync, nc.gpsimd
                    # x_t holds d in cols [0, N_ROWS), a in [N_ROWS, 2*N_ROWS)
                    x_t = tin_pool.tile([P, 2 * N_ROWS], f16, tag="x")
                    ld_eng.dma_start(
                        out=x_t[:kw], in_=xT[p0:p0 + kw].rearrange("k i r -> k (i r)")
                    )
                    d_t = x_t[:, 0:N_ROWS]
                    a_t = x_t[:, N_ROWS:2 * N_ROWS]

                    # o_t holds E in cols [0, N_ROWS), O in [N_ROWS, 2*N_ROWS)
                    o_t = tout_pool.tile([P, 2 * N_ROWS], f16, tag="o")
                    for c in range(NRCH):
                        r0 = c * RCHUNK
                        rs = slice(r0, r0 + RCHUNK)
                        psE = psum_pool.tile([P, RCHUNK], f32, tag="psE")
                        psO = psum_pool.tile([P, RCHUNK], f32, tag="psO")
                        nc.tensor.matmul(
                            psE[:pw], W[0][:kw, :pw], d_t[:kw, rs],
                            start=True, stop=False,
                        )
                        nc.tensor.matmul(
                            psE[:pw], W[1][:kw, :pw], a_t[:kw, rs],
                            start=False, stop=True,
                        )
                        nc.tensor.matmul(
                            psO[:pw], W[2][:kw, :pw], d_t[:kw, rs],
                            start=True, stop=False,
                        )
                        nc.tensor.matmul(
                            psO[:pw], W[3][:kw, :pw], a_t[:kw, rs],
                            start=False, stop=True,
                        )
                        # split the PSUM drains across ScalarE and VectorE
                        nc.scalar.copy(o_t[:pw, r0:r0 + RCHUNK], psE[:pw])
                        nc.vector.tensor_copy(
                            o_t[:pw, N_ROWS + r0:N_ROWS + r0 + RCHUNK], psO[:pw]
                        )
                    st_eng.dma_start(
                        out=oT[p0:p0 + pw].rearrange("k i r -> k (i r)"), in_=o_t[:pw]
                    )
    nc.compile()
    return nc


def _prep_inputs(details, approximation, scaling, scaling_rec):
    d = np.asarray(details, dtype=np.float32)
    a = np.asarray(approximation, dtype=np.float32)
    s = np.asarray(scaling, dtype=np.float64)
    sr = np.asarray(scaling_rec, dtype=np.float64)

    g = sr[::-1].copy()
    g[1::2] *= -1.0

    # banded weights W[k, p] = coef[k - p]
    wb_np = np.zeros((P, 4 * P), np.float16)
    kk = np.arange(P)[:, None]
    pp = np.arange(P)[None, :]
    diff = kk - pp
    for b, (filt, lo) in enumerate(((g, 0), (s, 0), (g, 1), (s, 1))):
        # even blocks (lo=0): coef[j] = filt[2j], j = k-p in [0, 3]
        # odd blocks (lo=1):  coef[j] = filt[2j+1], j = k-p-1 in [0, 3]
        j = diff - lo
        mask = (j >= 0) & (j < 4)
        vals = np.zeros((P, P), np.float32)
        vals[mask] = np.asarray(filt, np.float32)[2 * j[mask] + lo]
        wb_np[:, b * P:(b + 1) * P] = vals.astype(np.float16)

    # xT[u, 0, r] = d[r, u]; xT[u, 1, r] = a[r, u]  (u has 4 wrap columns)
    dTf = np.concatenate([d, d[:, :HALO]], axis=1).astype(np.float16).T
    aTf = np.concatenate([a, a[:, :HALO]], axis=1).astype(np.float16).T
    xT_np = np.ascontiguousarray(np.stack([dTf, aTf], axis=1))
    return xT_np, wb_np


def make_in_maps(details, approximation, scaling, scaling_rec):
    xT_np, wb_np = _prep_inputs(details, approximation, scaling, scaling_rec)
    in_maps = []
    for core in range(N_CORES):
        u0 = core * COLS_PER_CORE
        u1 = u0 + COLS_PER_CORE + HALO
        in_maps.append({"xT": xT_np[u0:u1], "wb": wb_np})
    return in_maps


def kernel(details, approximation, scaling, scaling_rec):
    if "nc" not in _CACHE:
        _CACHE["nc"] = _build()
    nc = _CACHE["nc"]

    from concourse.bass_utils import run_bass_kernel_spmd

    in_maps = make_in_maps(details, approximation, scaling, scaling_rec)
    res = run_bass_kernel_spmd(nc, in_maps, core_ids=list(range(N_CORES)))
    oT = np.concatenate([r["oT"] for r in res.results], axis=0)  # [M, 2, N_ROWS]
    out = np.empty((N_ROWS, 2 * M), np.float32)
    out[:, 0::2] = oT[:, 0].T
    out[:, 1::2] = oT[:, 1].T
    return out


# revision 14
# speedup vs baseline: 1.7641x; 1.0785x over previous
"""Trainium2 Bass kernel for nn_BackwardTransformLayer (inverse wavelet step).

Math (polyphase form of the reference):
    g = flip(scaling_rec); g[1::2] *= -1
    E[r, u] = sum_{j=0..3} g[2j]   * d[r, (u+j)   % M] + s[2j]   * a[r, (u+j)   % M]
    O[r, u] = sum_{j=0..3} g[2j+1] * d[r, (u+1+j) % M] + s[2j+1] * a[r, (u+1+j) % M]
    out[r, 2u] = E[r, u]; out[r, 2u+1] = O[r, u]

Harness tolerance is 2e-2, so everything runs in fp16 (rel err ~5e-4).

Layout trick: inputs are transposed HOST-side so the FIR axis u lies on SBUF
partitions.  A single matmul with a banded 128x128 weight matrix
W[k, p] = coef[k-p] computes a full 4-tap FIR for 124 output columns in one
pass over the moving tensor (rows in the free dim):

    psE[p, r] = sum_k Wd_e[k, p] * dT[k, r] + Wa_e[k, p] * aT[k, r]

Four matmuls per 124-column tile chunk (d/a x even/odd polyphase) do all 16
MACs/column, so PE covers the WHOLE problem (~62us/core).  ScalarE and
VectorE split the PSUM -> SBUF fp16 drains.  The kernel is DMA-bound.

DMA: d/a are interleaved host-side into xT[u, 2, r] and E/O outputs into
oT[u, 2, r], so each 128-column tile is ONE 2MB load and ONE 2MB store
(8KB contiguous lines).  Loads/stores alternate between the two HWDGE
rings (SP / ACT) per tile to split bytes evenly across both rings.

Sharding: embarrassingly parallel over columns u: 1024 columns per core
(+4 circular halo), all 4096 rows in the free dim.
"""

import numpy as np

P = 128
M = 8192                       # input columns (output cols = 2M interleaved)
N_ROWS = 4096
N_CORES = 8
COLS_PER_CORE = M // N_CORES   # 1024
HALO = 4                       # odd polyphase reaches k = p+4
STRIDE = P - HALO              # 124 valid output columns per 128-partition tile
RCHUNK = 512                   # PSUM bank capacity in f32
NRCH = N_ROWS // RCHUNK        # 8 row chunks
_CACHE = {}


def _tiles():
    """(col_offset, k_width, p_width) per tile covering [0, COLS_PER_CORE)."""
    out = []
    p0 = 0
    while p0 < COLS_PER_CORE:
        pw = min(STRIDE, COLS_PER_CORE - p0)
        kw = min(pw + HALO, P)
        out.append((p0, kw, pw))
        p0 += pw
    return out


def _build(reps=1):
    import contextlib

    import concourse.bacc as bacc
    import concourse.mybir as mybir
    from concourse.tile import TileContext

    f32 = mybir.dt.float32
    f16 = mybir.dt.float16

    nc = bacc.Bacc("TRN2", target_bir_lowering=False, debug=False, num_swdge_queues=2)
    xT = nc.dram_tensor(
        "xT", [COLS_PER_CORE + HALO, 2, N_ROWS], f16, kind="ExternalInput"
    )
    wb = nc.dram_tensor("wb", [P, 4 * P], f16, kind="ExternalInput")
    oT = nc.dram_tensor("oT", [COLS_PER_CORE, 2, N_ROWS], f16, kind="ExternalOutput")

    with TileContext(nc) as tc:
        with (
            tc.tile_pool(name="const", bufs=1) as const_pool,
            tc.tile_pool(name="tin", bufs=6) as tin_pool,
            tc.tile_pool(name="tout", bufs=4) as tout_pool,
            tc.tile_pool(name="psum", bufs=4, space="PSUM") as psum_pool,
        ):
            wb_sb = const_pool.tile([P, 4 * P], f16)
            nc.sync.dma_start(out=wb_sb[:], in_=wb[:])
            # weight blocks: 0=Wd_even 1=Wa_even 2=Wd_odd 3=Wa_odd
            W = [wb_sb[:, b * P:(b + 1) * P] for b in range(4)]

            rep_ctx = tc.For_i(0, reps, 1) if reps > 1 else contextlib.nullcontext()
            with rep_ctx:
                for ti, (p0, kw, pw) in enumerate(_tiles()):
                    # Loads on the SP HWDGE ring (SP runs nothing else);
                    # stores on the GPSIMD SWDGE ring (also otherwise idle).
                    # ACT/DVE only drain PSUM, so no DMA trigger ever queues
                    # behind a compute wait.
                    ld_eng, st_eng = nc.sync, nc.gpsimd
                    # x_t holds d in cols [0, N_ROWS), a in [N_ROWS, 2*N_ROWS)
                    x_t = tin_pool.tile([P, 2 * N_ROWS], f16, tag="x")
                    ld_eng.dma_start(
                        out=x_t[:kw], in_=xT[p0:p0 + kw].rearrange("k i r -> k (i r)")
                    )
                    d_t = x_t[:, 0:N_ROWS]
                    a_t = x_t[:, N_ROWS:2 * N_ROWS]

                    # o_t holds E in cols [0, N_ROWS), O in [N_ROWS, 2*N_ROWS)
                    o_t = tout_pool.tile([P, 2 * N_ROWS], f16, tag="o")
                    for c in range(NRCH):
                        r0 = c * RCHUNK
                        rs = slice(r0, r0 + RCHUNK)
                        psE = psum_pool.tile([P, RCHUNK], f32, tag="psE")
                        psO = psum_pool.tile([P, RCHUNK], f32, tag="psO")
                        nc.tensor.matmul(
                            psE[:pw], W[0][:kw, :pw], d_t[:kw, rs],
                            start=True, stop=False,
                        )
                        nc.tensor.matmul(
                            psE[:pw], W[1][:kw, :pw], a_t[:kw, rs],
                            start=False, stop=True,
                        )
                        nc.tensor.matmul(
                            psO[:pw], W[2][:kw, :pw], d_t[:kw, rs],
                            start=True, stop=False,
                        )
                        nc.tensor.matmul(
                            psO[:pw], W[3][:kw, :pw], a_t[:kw, rs],
                            start=False, stop=True,
                        )
                        # split the PSUM drains across ScalarE and VectorE
                        nc.scalar.copy(o_t[:pw, r0:r0 + RCHUNK], psE[:pw])
                        nc.vector.tensor_copy(
                            o_t[:pw, N_ROWS + r0:N_ROWS + r0 + RCHUNK], psO[:pw]
                        )
                    # Store the two polyphase halves as separate DMAs on the
                    # two SWDGE queues: each half's dependency is a single
                    # drain engine (E=ScalarE, O=VectorE), so each can start
                    # as soon as its own drains finish, and the two queues'
                    # transfers overlap.
                    oout = oT[p0:p0 + pw].rearrange("k i r -> k (i r)")
                    st_eng.dma_start(out=oout[:, 0:N_ROWS], in_=o_t[:pw, 0:N_ROWS])
                    st2 = st_eng.dma_start(
                        out=oout[:, N_ROWS:], in_=o_t[:pw, N_ROWS:]
                    )
                    st2.ins.queue = "qPoolDynamic1"
    nc.compile()
    return nc


def _prep_inputs(details, approximation, scaling, scaling_rec):
    d = np.asarray(details, dtype=np.float32)
    a = np.asarray(approximation, dtype=np.float32)
    s = np.asarray(scaling, dtype=np.float64)
    sr = np.asarray(scaling_rec, dtype=np.float64)

    g = sr[::-1].copy()
    g[1::2] *= -1.0

    # banded weights W[k, p] = coef[k - p]
    wb_np = np.zeros((P, 4 * P), np.float16)
    kk = np.arange(P)[:, None]
    pp = np.arange(P)[None, :]
    diff = kk - pp
    for b, (filt, lo) in enumerate(((g, 0), (s, 0), (g, 1), (s, 1))):
        # even blocks (lo=0): coef[j] = filt[2j], j = k-p in [0, 3]
        # odd blocks (lo=1):  coef[j] = filt[2j+1], j = k-p-1 in [0, 3]
        j = diff - lo
        mask = (j >= 0) & (j < 4)
        vals = np.zeros((P, P), np.float32)
        vals[mask] = np.asarray(filt, np.float32)[2 * j[mask] + lo]
        wb_np[:, b * P:(b + 1) * P] = vals.astype(np.float16)

    # xT[u, 0, r] = d[r, u]; xT[u, 1, r] = a[r, u]  (u has 4 wrap columns)
    dTf = np.concatenate([d, d[:, :HALO]], axis=1).astype(np.float16).T
    aTf = np.concatenate([a, a[:, :HALO]], axis=1).astype(np.float16).T
    xT_np = np.ascontiguousarray(np.stack([dTf, aTf], axis=1))
    return xT_np, wb_np


def make_in_maps(details, approximation, scaling, scaling_rec):
    xT_np, wb_np = _prep_inputs(details, approximation, scaling, scaling_rec)
    in_maps = []
    for core in range(N_CORES):
        u0 = core * COLS_PER_CORE
        u1 = u0 + COLS_PER_CORE + HALO
        in_maps.append({"xT": xT_np[u0:u1], "wb": wb_np})
    return in_maps


def kernel(details, approximation, scaling, scaling_rec):
    if "nc" not in _CACHE:
        _CACHE["nc"] = _build()
    nc = _CACHE["nc"]

    from concourse.bass_utils import run_bass_kernel_spmd

    in_maps = make_in_maps(details, approximation, scaling, scaling_rec)
    res = run_bass_kernel_spmd(nc, in_maps, core_ids=list(range(N_CORES)))
    oT = np.concatenate([r["oT"] for r in res.results], axis=0)  # [M, 2, N_ROWS]
    out = np.empty((N_ROWS, 2 * M), np.float32)
    out[:, 0::2] = oT[:, 0].T
    out[:, 1::2] = oT[:, 1].T
    return out


# revision 16
# speedup vs baseline: 1.8697x; 1.0598x over previous
"""Trainium2 Bass kernel for nn_BackwardTransformLayer (inverse wavelet step).

Math (polyphase form of the reference):
    g = flip(scaling_rec); g[1::2] *= -1
    E[r, u] = sum_{j=0..3} g[2j]   * d[r, (u+j)   % M] + s[2j]   * a[r, (u+j)   % M]
    O[r, u] = sum_{j=0..3} g[2j+1] * d[r, (u+1+j) % M] + s[2j+1] * a[r, (u+1+j) % M]
    out[r, 2u] = E[r, u]; out[r, 2u+1] = O[r, u]

Harness tolerance is 2e-2, so everything runs in fp16 (rel err ~5e-4).

Layout trick: inputs are transposed HOST-side so the FIR axis u lies on SBUF
partitions.  A single matmul with a banded 128x128 weight matrix
W[k, p] = coef[k-p] computes a full 4-tap FIR for 124 output columns in one
pass over the moving tensor (rows in the free dim):

    psE[p, r] = sum_k Wd_e[k, p] * dT[k, r] + Wa_e[k, p] * aT[k, r]

Four matmuls per 124-column tile chunk (d/a x even/odd polyphase) do all 16
MACs/column, so PE covers the WHOLE problem (~62us/core).  ScalarE and
VectorE split the PSUM -> SBUF fp16 drains.  The kernel is DMA-bound.

DMA: d/a are interleaved host-side into xT[u, 2, r] and E/O outputs into
oT[u, 2, r], so each 128-column tile is ONE 2MB load and ONE 2MB store
(8KB contiguous lines).  Loads/stores alternate between the two HWDGE
rings (SP / ACT) per tile to split bytes evenly across both rings.

Sharding: embarrassingly parallel over columns u: 1024 columns per core
(+4 circular halo), all 4096 rows in the free dim.
"""

import numpy as np

P = 128
M = 8192                       # input columns (output cols = 2M interleaved)
N_ROWS = 4096
N_CORES = 8
COLS_PER_CORE = M // N_CORES   # 1024
HALO = 4                       # odd polyphase reaches k = p+4
STRIDE = P - HALO              # 124 valid output columns per 128-partition tile
RCHUNK = 512                   # PSUM bank capacity in f32
NRCH = N_ROWS // RCHUNK        # 8 row chunks
_CACHE = {}


def _tiles():
    """(col_offset, k_width, p_width) per tile covering [0, COLS_PER_CORE)."""
    out = []
    p0 = 0
    while p0 < COLS_PER_CORE:
        pw = min(STRIDE, COLS_PER_CORE - p0)
        kw = min(pw + HALO, P)
        out.append((p0, kw, pw))
        p0 += pw
    return out


def _build(reps=1):
    import contextlib

    import concourse.bacc as bacc
    import concourse.mybir as mybir
    from concourse.tile import TileContext

    f32 = mybir.dt.float32
    f16 = mybir.dt.float16

    nc = bacc.Bacc("TRN2", target_bir_lowering=False, debug=False, num_swdge_queues=2)
    xT = nc.dram_tensor(
        "xT", [COLS_PER_CORE + HALO, 2, N_ROWS], f16, kind="ExternalInput"
    )
    wb = nc.dram_tensor("wb", [P, 4 * P], f16, kind="ExternalInput")
    oT = nc.dram_tensor("oT", [COLS_PER_CORE, 2, N_ROWS], f16, kind="ExternalOutput")

    with TileContext(nc) as tc:
        with (
            tc.tile_pool(name="const", bufs=1) as const_pool,
            tc.tile_pool(name="tin", bufs=6) as tin_pool,
            tc.tile_pool(name="tout", bufs=4) as tout_pool,
            tc.tile_pool(name="psum", bufs=4, space="PSUM") as psum_pool,
        ):
            wb_sb = const_pool.tile([P, 4 * P], f16)
            nc.sync.dma_start(out=wb_sb[:], in_=wb[:])
            # weight blocks: 0=Wd_even 1=Wa_even 2=Wd_odd 3=Wa_odd
            W = [wb_sb[:, b * P:(b + 1) * P] for b in range(4)]

            rep_ctx = tc.For_i(0, reps, 1) if reps > 1 else contextlib.nullcontext()
            with rep_ctx:
                for ti, (p0, kw, pw) in enumerate(_tiles()):
                    # Loads on the SP HWDGE ring (SP runs nothing else);
                    # stores on the GPSIMD SWDGE ring (also otherwise idle).
                    # ACT/DVE only drain PSUM, so no DMA trigger ever queues
                    # behind a compute wait.
                    ld_eng, st_eng = nc.sync, nc.gpsimd
                    # x_t holds d in cols [0, N_ROWS), a in [N_ROWS, 2*N_ROWS)
                    x_t = tin_pool.tile([P, 2 * N_ROWS], f16, tag="x")
                    ld_eng.dma_start(
                        out=x_t[:kw], in_=xT[p0:p0 + kw].rearrange("k i r -> k (i r)")
                    )
                    d_t = x_t[:, 0:N_ROWS]
                    a_t = x_t[:, N_ROWS:2 * N_ROWS]

                    # o_t holds E in cols [0, N_ROWS), O in [N_ROWS, 2*N_ROWS)
                    o_t = tout_pool.tile([P, 2 * N_ROWS], f16, tag="o")
                    for c in range(NRCH):
                        r0 = c * RCHUNK
                        rs = slice(r0, r0 + RCHUNK)
                        psE = psum_pool.tile([P, RCHUNK], f32, tag="psE")
                        psO = psum_pool.tile([P, RCHUNK], f32, tag="psO")
                        nc.tensor.matmul(
                            psE[:pw], W[0][:kw, :pw], d_t[:kw, rs],
                            start=True, stop=False,
                        )
                        nc.tensor.matmul(
                            psE[:pw], W[1][:kw, :pw], a_t[:kw, rs],
                            start=False, stop=True,
                        )
                        nc.tensor.matmul(
                            psO[:pw], W[2][:kw, :pw], d_t[:kw, rs],
                            start=True, stop=False,
                        )
                        nc.tensor.matmul(
                            psO[:pw], W[3][:kw, :pw], a_t[:kw, rs],
                            start=False, stop=True,
                        )
                        # split the PSUM drains across ScalarE and VectorE
                        nc.scalar.copy(o_t[:pw, r0:r0 + RCHUNK], psE[:pw])
                        nc.vector.tensor_copy(
                            o_t[:pw, N_ROWS + r0:N_ROWS + r0 + RCHUNK], psO[:pw]
                        )
                    # Store the two polyphase halves as separate DMAs on the
                    # two SWDGE queues: each half's dependency is a single
                    # drain engine (E=ScalarE, O=VectorE), so each can start
                    # as soon as its own drains finish, and the two queues'
                    # transfers overlap.
                    oout = oT[p0:p0 + pw].rearrange("k i r -> k (i r)")
                    st_eng.dma_start(out=oout[:, 0:N_ROWS], in_=o_t[:pw, 0:N_ROWS])
                    st2 = st_eng.dma_start(
                        out=oout[:, N_ROWS:], in_=o_t[:pw, N_ROWS:]
                    )
                    st2.ins.queue = "qPoolDynamic1"
    nc.compile()
    return nc


def _prep_inputs(details, approximation, scaling, scaling_rec):
    d = np.asarray(details, dtype=np.float32)
    a = np.asarray(approximation, dtype=np.float32)
    s = np.asarray(scaling, dtype=np.float64)
    sr = np.asarray(scaling_rec, dtype=np.float64)

    g = sr[::-1].copy()
    g[1::2] *= -1.0

    # banded weights W[k, p] = coef[k - p]
    wb_np = np.zeros((P, 4 * P), np.float16)
    kk = np.arange(P)[:, None]
    pp = np.arange(P)[None, :]
    diff = kk - pp
    for b, (filt, lo) in enumerate(((g, 0), (s, 0), (g, 1), (s, 1))):
        # even blocks (lo=0): coef[j] = filt[2j], j = k-p in [0, 3]
        # odd blocks (lo=1):  coef[j] = filt[2j+1], j = k-p-1 in [0, 3]
        j = diff - lo
        mask = (j >= 0) & (j < 4)
        vals = np.zeros((P, P), np.float32)
        vals[mask] = np.asarray(filt, np.float32)[2 * j[mask] + lo]
        wb_np[:, b * P:(b + 1) * P] = vals.astype(np.float16)

    # xT[u, 0, r] = d[r, u]; xT[u, 1, r] = a[r, u]  (u has 4 wrap columns)
    dTf = np.concatenate([d, d[:, :HALO]], axis=1).astype(np.float16).T
    aTf = np.concatenate([a, a[:, :HALO]], axis=1).astype(np.float16).T
    xT_np = np.ascontiguousarray(np.stack([dTf, aTf], axis=1))
    return xT_np, wb_np


def make_in_maps(details, approximation, scaling, scaling_rec):
    xT_np, wb_np = _prep_inputs(details, approximation, scaling, scaling_rec)
    in_maps = []
    for core in range(N_CORES):
        u0 = core * COLS_PER_CORE
        u1 = u0 + COLS_PER_CORE + HALO
        in_maps.append({"xT": xT_np[u0:u1], "wb": wb_np})
    return in_maps


def kernel(details, approximation, scaling, scaling_rec):
    if "nc" not in _CACHE:
        _CACHE["nc"] = _build()
    nc = _CACHE["nc"]

    from concourse.bass_utils import run_bass_kernel_spmd

    in_maps = make_in_maps(details, approximation, scaling, scaling_rec)
    res = run_bass_kernel_spmd(nc, in_maps, core_ids=list(range(N_CORES)))
    oT = np.concatenate([r["oT"] for r in res.results], axis=0)  # [M, 2, N_ROWS]
    out = np.empty((N_ROWS, 2 * M), np.float32)
    out[:, 0::2] = oT[:, 0].T
    out[:, 1::2] = oT[:, 1].T
    return out


# revision 17
# speedup vs baseline: 5.4807x; 2.9314x over previous
"""Trainium2 Bass kernel for nn_BackwardTransformLayer (inverse wavelet step).

Math (polyphase form of the reference):
    g = flip(scaling_rec); g[1::2] *= -1
    E[r, u] = sum_{j=0..3} g[2j]   * d[r, (u+j)   % M] + s[2j]   * a[r, (u+j)   % M]
    O[r, u] = sum_{j=0..3} g[2j+1] * d[r, (u+1+j) % M] + s[2j+1] * a[r, (u+1+j) % M]
    out[r, 2u] = E[r, u]; out[r, 2u+1] = O[r, u]

Harness tolerance is 2e-2, so everything runs in fp16 (rel err ~5e-4).

Layout trick: inputs are transposed HOST-side so the FIR axis u lies on SBUF
partitions.  A single matmul with a banded 128x128 weight matrix
W[k, p] = coef[k-p] computes a full 4-tap FIR for 124 output columns in one
pass over the moving tensor (rows in the free dim):

    psE[p, r] = sum_k Wd_e[k, p] * dT[k, r] + Wa_e[k, p] * aT[k, r]

Four matmuls per 124-column tile chunk (d/a x even/odd polyphase) do all 16
MACs/column, so PE covers the WHOLE problem (~62us/core).  ScalarE and
VectorE split the PSUM -> SBUF fp16 drains.  The kernel is DMA-bound.

DMA: d/a are interleaved host-side into xT[u, 2, r] and E/O outputs into
oT[u, 2, r], so each 128-column tile is ONE 2MB load and ONE 2MB store
(8KB contiguous lines).  Loads/stores alternate between the two HWDGE
rings (SP / ACT) per tile to split bytes evenly across both rings.

Sharding: embarrassingly parallel over columns u: 1024 columns per core
(+4 circular halo), all 4096 rows in the free dim.
"""

import numpy as np

P = 128
M = 8192                       # input columns (output cols = 2M interleaved)
N_ROWS = 4096
N_CORES = 8
COLS_PER_CORE = M // N_CORES   # 1024
HALO = 4                       # odd polyphase reaches k = p+4
STRIDE = P - HALO              # 124 valid output columns per 128-partition tile
RCHUNK = 512                   # PSUM bank capacity in f32
NRCH = N_ROWS // RCHUNK        # 8 row chunks
_CACHE = {}


def _tiles():
    """(col_offset, k_width, p_width) per tile covering [0, COLS_PER_CORE)."""
    out = []
    p0 = 0
    while p0 < COLS_PER_CORE:
        pw = min(STRIDE, COLS_PER_CORE - p0)
        kw = min(pw + HALO, P)
        out.append((p0, kw, pw))
        p0 += pw
    return out


def _build(reps=1):
    import contextlib

    import concourse.bacc as bacc
    import concourse.mybir as mybir
    from concourse.tile import TileContext

    f32 = mybir.dt.float32
    f16 = mybir.dt.float16

    nc = bacc.Bacc("TRN2", target_bir_lowering=False, debug=False, num_swdge_queues=2)
    xT = nc.dram_tensor(
        "xT", [COLS_PER_CORE + HALO, 2, N_ROWS], f16, kind="ExternalInput"
    )
    wb = nc.dram_tensor("wb", [P, 4 * P], f16, kind="ExternalInput")
    oT = nc.dram_tensor("oT", [COLS_PER_CORE, 2, N_ROWS], f16, kind="ExternalOutput")

    with TileContext(nc) as tc:
        with (
            tc.tile_pool(name="const", bufs=1) as const_pool,
            tc.tile_pool(name="tin", bufs=6) as tin_pool,
            tc.tile_pool(name="tout", bufs=4) as tout_pool,
            tc.tile_pool(name="psum", bufs=4, space="PSUM") as psum_pool,
        ):
            wb_sb = const_pool.tile([P, 4 * P], f16)
            nc.sync.dma_start(out=wb_sb[:], in_=wb[:])
            # weight blocks: 0=Wd_even 1=Wa_even 2=Wd_odd 3=Wa_odd
            W = [wb_sb[:, b * P:(b + 1) * P] for b in range(4)]

            rep_ctx = tc.For_i(0, reps, 1) if reps > 1 else contextlib.nullcontext()
            with rep_ctx:
                for ti, (p0, kw, pw) in enumerate(_tiles()):
                    # Loads on the SP HWDGE ring (SP runs nothing else);
                    # stores on the GPSIMD SWDGE ring (also otherwise idle).
                    # ACT/DVE only drain PSUM, so no DMA trigger ever queues
                    # behind a compute wait.
                    ld_eng, st_eng = nc.sync, nc.gpsimd
                    # x_t holds d in cols [0, N_ROWS), a in [N_ROWS, 2*N_ROWS)
                    x_t = tin_pool.tile([P, 2 * N_ROWS], f16, tag="x")
                    ld_eng.dma_start(
                        out=x_t[:kw], in_=xT[p0:p0 + kw].rearrange("k i r -> k (i r)")
                    )
                    d_t = x_t[:, 0:N_ROWS]
                    a_t = x_t[:, N_ROWS:2 * N_ROWS]

                    # o_t holds E in cols [0, N_ROWS), O in [N_ROWS, 2*N_ROWS)
                    o_t = tout_pool.tile([P, 2 * N_ROWS], f16, tag="o")
                    for c in range(NRCH):
                        r0 = c * RCHUNK
                        rs = slice(r0, r0 + RCHUNK)
                        psE = psum_pool.tile([P, RCHUNK], f32, tag="psE")
                        psO = psum_pool.tile([P, RCHUNK], f32, tag="psO")
                        # O before E: the O drain runs on the slower engine
                        # (VectorE), so giving it a per-chunk head start pulls
                        # the O-half store launch earlier.
                        nc.tensor.matmul(
                            psO[:pw], W[2][:kw, :pw], d_t[:kw, rs],
                            start=True, stop=False,
                        )
                        nc.tensor.matmul(
                            psO[:pw], W[3][:kw, :pw], a_t[:kw, rs],
                            start=False, stop=True,
                        )
                        nc.tensor.matmul(
                            psE[:pw], W[0][:kw, :pw], d_t[:kw, rs],
                            start=True, stop=False,
                        )
                        nc.tensor.matmul(
                            psE[:pw], W[1][:kw, :pw], a_t[:kw, rs],
                            start=False, stop=True,
                        )
                        # split the PSUM drains across ScalarE and VectorE
                        nc.scalar.copy(o_t[:pw, r0:r0 + RCHUNK], psE[:pw])
                        nc.vector.tensor_copy(
                            o_t[:pw, N_ROWS + r0:N_ROWS + r0 + RCHUNK], psO[:pw]
                        )
                    # Store the two polyphase halves as separate DMAs on the
                    # two SWDGE queues: each half's dependency is a single
                    # drain engine (E=ScalarE, O=VectorE), so each can start
                    # as soon as its own drains finish, and the two queues'
                    # transfers overlap.
                    oout = oT[p0:p0 + pw].rearrange("k i r -> k (i r)")
                    st_eng.dma_start(out=oout[:, 0:N_ROWS], in_=o_t[:pw, 0:N_ROWS])
                    st2 = st_eng.dma_start(
                        out=oout[:, N_ROWS:], in_=o_t[:pw, N_ROWS:]
                    )
                    st2.ins.queue = "qPoolDynamic1"
    nc.compile()
    return nc


def _prep_inputs(details, approximation, scaling, scaling_rec):
    d = np.asarray(details, dtype=np.float32)
    a = np.asarray(approximation, dtype=np.float32)
    s = np.asarray(scaling, dtype=np.float64)
    sr = np.asarray(scaling_rec, dtype=np.float64)

    g = sr[::-1].copy()
    g[1::2] *= -1.0

    # banded weights W[k, p] = coef[k - p]
    wb_np = np.zeros((P, 4 * P), np.float16)
    kk = np.arange(P)[:, None]
    pp = np.arange(P)[None, :]
    diff = kk - pp
    for b, (filt, lo) in enumerate(((g, 0), (s, 0), (g, 1), (s, 1))):
        # even blocks (lo=0): coef[j] = filt[2j], j = k-p in [0, 3]
        # odd blocks (lo=1):  coef[j] = filt[2j+1], j = k-p-1 in [0, 3]
        j = diff - lo
        mask = (j >= 0) & (j < 4)
        vals = np.zeros((P, P), np.float32)
        vals[mask] = np.asarray(filt, np.float32)[2 * j[mask] + lo]
        wb_np[:, b * P:(b + 1) * P] = vals.astype(np.float16)

    # xT[u, 0, r] = d[r, u]; xT[u, 1, r] = a[r, u]  (u has 4 wrap columns)
    dTf = np.concatenate([d, d[:, :HALO]], axis=1).astype(np.float16).T
    aTf = np.concatenate([a, a[:, :HALO]], axis=1).astype(np.float16).T
    xT_np = np.ascontiguousarray(np.stack([dTf, aTf], axis=1))
    return xT_np, wb_np


def make_in_maps(details, approximation, scaling, scaling_rec):
    xT_np, wb_np = _prep_inputs(details, approximation, scaling, scaling_rec)
    in_maps = []
    for core in range(N_CORES):
        u0 = core * COLS_PER_CORE
        u1 = u0 + COLS_PER_CORE + HALO
        in_maps.append({"xT": xT_np[u0:u1], "wb": wb_np})
    return in_maps


def kernel(details, approximation, scaling, scaling_rec):
    if "nc" not in _CACHE:
        _CACHE["nc"] = _build()
    nc = _CACHE["nc"]

    from concourse.bass_utils import run_bass_kernel_spmd

    in_maps = make_in_maps(details, approximation, scaling, scaling_rec)
    res = run_bass_kernel_spmd(nc, in_maps, core_ids=list(range(N_CORES)))
    oT = np.concatenate([r["oT"] for r in res.results], axis=0)  # [M, 2, N_ROWS]
    out = np.empty((N_ROWS, 2 * M), np.float32)
    out[:, 0::2] = oT[:, 0].T
    out[:, 1::2] = oT[:, 1].T
    return out
